# revision 39
# baseline (speedup 1.0000x reference)
"""DIN attention layer kernel for Trainium2 (8 NeuronCores, data-parallel over batch).

Reference math (per batch b, t in [0,200), E=36):
  x  = concat([q, ub, q-ub, q*ub], -1)             # [B,T,144]
  h1 = sigmoid(x @ W1 + b1)                        # [B,T,80]
  h2 = sigmoid(h1 @ W2 + b2)                       # [B,T,40]
  s  = h2 @ W3 + b3                                # [B,T,1]
  w  = softmax(s.T * mask)                         # [B,1,T]  (multiplicative mask)
  out = w @ ub                                     # [B,1,36]

Host-side algebraic folds:
  1) x @ W1 = ub @ (Wb-Wc) + (q*ub) @ Wd + q @ (Wa+Wc); q is per-batch, so fold
     into per-batch weights waug_b = [(Wb-Wc) + diag(q_b) Wd ; q_b(Wa+Wc)+b1]
     ([37,80]) and augment ub with a ones column -> single K=37 matmul.
  2) sigmoid(x) = 0.5 + 0.5*tanh(x/2); the 0.5/0.5 affine is folded into the
     next layer's weights/biases, so the device only evaluates tanh.

Device dataflow (per core, bc=512 batches, chunk = 4 batches). All matmul
operands live at SBUF base_partition 0 (row-offset operands fault on HW):
  - mm1: batch PAIRS stacked in the contraction dim ([74, 80] stationary,
    rhs zero-padded complementarily) -> one LDWEIGHTS + one N=400 matmul
    per 2 batches.
  - mm2: shared [80, 40] weights, two col-tiled instances -> [104, 400]
    PSUM (different batches on partitions 0:40 / 64:104).
  - mm3: two zero-padded [104, 1] stationaries selecting a partition half;
    each N=400 matmul yields 2 batches' scores.
  - The PE instruction stream is software-pipelined (mm1(k), mm2(k-1),
    mm3(k-2)) so the PE never waits on ACT results and vice versa.
  - scores + b3 are copied PSUM->SBUF on DVE, DMA partition-scattered into a
    [128 batches, 200] tile; softmax there (DVE + one ACT exp).
  - out = w @ ub computed on DVE: broadcast-multiply + axis reduce against a
    host-provided batch-on-partition copy of ub.
  - Bulk loads are issued from GPSIMD (software DGE) split into 8-partition
    pieces so they round-robin across the 16 DMA engines.
"""

from contextlib import ExitStack

import numpy as np
import ml_dtypes

import concourse.bass as bass
import concourse.bacc as bacc
import concourse.tile as tile
from concourse import mybir
from concourse.bass_utils import run_bass_kernel_spmd

B, T, E = 4096, 200, 36
N_CORES = 8
BC = B // N_CORES          # batches per core
F32 = mybir.dt.float32
BF16 = mybir.dt.bfloat16
AF = mybir.ActivationFunctionType
BF16NP = ml_dtypes.bfloat16

N_GRP = BC // 128          # softmax/wsum groups of 128 batches
NK = BC // 4               # chunks of 4 batches


def build_module():
    bc = BC
    nc = bacc.Bacc(
        "TRN2", target_bir_lowering=False, debug=False,
        enable_asserts=False, num_devices=N_CORES,
    )

    # host-prepped DRAM inputs (bf16 unless noted)
    ubt_d = nc.dram_tensor("ubt", [74, bc * 200], BF16, kind="ExternalInput").ap()
    wg_d = nc.dram_tensor("wg", [74, bc * 40], BF16, kind="ExternalInput").ap()
    ubbw_d = nc.dram_tensor("ubbw", [128, N_GRP * 36 * 200], BF16, kind="ExternalInput").ap()
    lens_d = nc.dram_tensor("lens", [128, N_GRP], F32, kind="ExternalInput").ap()
    w2_d = nc.dram_tensor("w2", [80, 40], BF16, kind="ExternalInput").ap()
    w3_d = nc.dram_tensor("w3", [104, 2], BF16, kind="ExternalInput").ap()
    b2c_d = nc.dram_tensor("b2c", [128, 1], F32, kind="ExternalInput").ap()
    b3c_d = nc.dram_tensor("b3c", [128, 1], F32, kind="ExternalInput").ap()
    out_d = nc.dram_tensor("out", [bc, 36], F32, kind="ExternalOutput").ap()

    iota_d = nc.inline_tensor(
        np.broadcast_to(np.arange(200, dtype=np.float32), (128, 200)).copy(),
        name="iotat").ap()

    with tile.TileContext(nc) as tc, ExitStack() as es:
        cpool = es.enter_context(tc.tile_pool(name="consts", bufs=1))
        ubtp = es.enter_context(tc.tile_pool(name="ubtp", bufs=2))
        wgp = es.enter_context(tc.tile_pool(name="wgp", bufs=2))
        h1p = es.enter_context(tc.tile_pool(name="h1p", bufs=3))
        h2p = es.enter_context(tc.tile_pool(name="h2p", bufs=3))
        scp = es.enter_context(tc.tile_pool(name="scp", bufs=2))
        scgp = es.enter_context(tc.tile_pool(name="scgp", bufs=2))
        ubbp = es.enter_context(tc.tile_pool(name="ubbp", bufs=2))
        prodp = es.enter_context(tc.tile_pool(name="prodp", bufs=2))
        smp = es.enter_context(tc.tile_pool(name="smp", bufs=2))
        outp = es.enter_context(tc.tile_pool(name="outp", bufs=2))
        m1p = es.enter_context(tc.tile_pool(name="m1p", bufs=1, space="PSUM"))
        m2p = es.enter_context(tc.tile_pool(name="m2p", bufs=1, space="PSUM"))
        m3p = es.enter_context(tc.tile_pool(name="m3p", bufs=1, space="PSUM"))

        iota_t = cpool.tile([128, 200], F32)
        nc.sync.dma_start(out=iota_t, in_=iota_d)
        w2_t = cpool.tile([80, 40], BF16)
        nc.sync.dma_start(out=w2_t, in_=w2_d)
        w3_t = cpool.tile([104, 2], BF16)
        nc.sync.dma_start(out=w3_t, in_=w3_d)
        b2_t = cpool.tile([128, 1], F32)
        nc.sync.dma_start(out=b2_t, in_=b2c_d)
        b3_t = cpool.tile([128, 1], F32)
        nc.sync.dma_start(out=b3_t, in_=b3c_d)
        lens_t = cpool.tile([128, N_GRP], F32)
        nc.sync.dma_start(out=lens_t, in_=lens_d)

        # persistent double-buffered PSUM tiles (2x2 + 2x1 + 2x1 = 8 banks);
        # mm2/mm3 leave partition gaps that later full-partition reads
        # touch, so zero once up front
        m1s = [m1p.tile([80, 1024], F32, name=f"m1_{i}") for i in range(2)]
        m2s = [m2p.tile([104, 512], F32, name=f"m2_{i}") for i in range(2)]
        nc.vector.memset(m2s[0], 0.0)
        nc.vector.memset(m2s[1], 0.0)
        m3s = [m3p.tile([128, 400], F32, name=f"m3_{i}") for i in range(2)]
        nc.vector.memset(m3s[0], 0.0)
        nc.vector.memset(m3s[1], 0.0)

        ubt2_t = wg2_t = None
        h1s, h2s, scgs, ubbs = {}, {}, {}, {}

        wsum_pending = []

        def softmax_wsum(g):
            scg_t = scgs.pop(g)
            ubb_t = ubbs.pop(g)
            mask_t = smp.tile([128, 200], F32, tag="mask")
            nc.vector.tensor_scalar(
                out=mask_t, in0=iota_t, scalar1=lens_t[:, g:g + 1],
                scalar2=None, op0=mybir.AluOpType.is_lt)
            masked = smp.tile([128, 200], F32, tag="masked")
            nc.vector.tensor_mul(masked, scg_t, mask_t)
            negmax = smp.tile([128, 1], F32, tag="negmax")
            nc.vector.tensor_reduce(
                out=negmax, in_=masked, axis=mybir.AxisListType.X,
                op=mybir.AluOpType.max, negate=True)
            ew = smp.tile([128, 200], F32, tag="ew")
            sumexp = smp.tile([128, 1], F32, tag="sumexp")
            nc.scalar.activation(
                out=ew, in_=masked, func=AF.Exp,
                bias=negmax, accum_out=sumexp)
            rz = smp.tile([128, 1], F32, tag="rz")
            nc.vector.reciprocal(rz, sumexp)
            w_t = smp.tile([128, 200], BF16, tag="wt")
            nc.vector.tensor_scalar_mul(w_t, ew, rz)
            # out[p, e] = sum_t w[p, t] * ub[p, e, t], deferred in 4
            # e-slices so the DVE never blocks the score-drain path
            prod_t = prodp.tile([128, 36 * 200], BF16, tag="prod")
            out_t = outp.tile([128, 36], F32, tag="out")

            def piece(i):
                def emit():
                    e0, e1 = 9 * i, 9 * i + 9
                    nc.vector.tensor_tensor(
                        out=prod_t.rearrange("p (e t) -> p e t", e=36)[:, e0:e1],
                        in0=ubb_t.rearrange("p (e t) -> p e t", e=36)[:, e0:e1],
                        in1=bass.AP(tensor=w_t.tensor, offset=w_t.offset,
                                    ap=[[200, 128], [0, 9], [1, 200]]),
                        op=mybir.AluOpType.mult)
                    nc.vector.tensor_reduce(
                        out=out_t[:, e0:e1],
                        in_=prod_t.rearrange("p (e t) -> p e t", e=36)[:, e0:e1],
                        axis=mybir.AxisListType.X, op=mybir.AluOpType.add)
                    if i == 3:
                        nc.sync.dma_start(
                            out=bass.AP(tensor=out_d.tensor,
                                        offset=out_d.offset + g * 128 * 36,
                                        ap=[[36, 128], [1, 36]]),
                            in_=out_t)
                return emit
            for i in range(4):
                wsum_pending.append(piece(i))

        for kk in range(NK + 10):
            if kk < NK:
                k = kk
                ph, php = k // 8, (k // 8) % 4
                if k % 16 == 0:
                    # phase-pair loads, split by partition range and spread
                    # over three DMA issue paths: GPSIMD software-DGE pieces
                    # round-robin across DMA engines; the SP and ACT hardware
                    # DGE rings carry a share each in parallel
                    ubt2_t = ubtp.tile([74, 12800], BF16, tag="ubt")
                    wg2_t = wgp.tile([74, 2560], BF16, tag="wg")
                    csplit = 1
                    cw = 12800 // csplit
                    for ci in range(csplit):
                        for p0 in range(0, 74, 5):
                            p1 = min(74, p0 + 5)
                            nc.gpsimd.dma_start(
                                out=ubt2_t[p0:p1, ci * cw:(ci + 1) * cw],
                                in_=bass.AP(tensor=ubt_d.tensor,
                                            offset=ubt_d.offset + ph * 6400
                                            + p0 * (bc * 200) + ci * cw,
                                            ap=[[bc * 200, p1 - p0], [1, cw]]))
                    for p0 in range(0, 74, 16):
                        p1 = min(74, p0 + 16)
                        nc.gpsimd.dma_start(
                            out=wg2_t[p0:p1, :],
                            in_=bass.AP(tensor=wg_d.tensor,
                                        offset=wg_d.offset + ph * 1280
                                        + p0 * (bc * 40),
                                        ap=[[bc * 40, p1 - p0], [1, 2560]]))
                if k % 32 == 0:
                    g = k // 32
                    ubbs[g] = ubbp.tile([128, 36 * 200], BF16, tag="ubb", name="ubb_t")
                    scgs[g] = scgp.tile([128, 200], F32, tag="scg",
                                        name="scg_t")
                if k % 8 == 4:
                    # quarter-loads of the group's behaviors, spread across
                    # the group period to avoid bursty DMA contention
                    g, qtr = k // 32, (k // 8) % 4
                    nc.sync.dma_start(
                        out=ubbs[g][:, qtr * 1800:(qtr + 1) * 1800],
                        in_=bass.AP(tensor=ubbw_d.tensor,
                                    offset=ubbw_d.offset + g * 36 * 200
                                    + qtr * 1800,
                                    ap=[[N_GRP * 36 * 200, 128], [1, 1800]]))
                # ---- mm1: 2 batch-pair matmuls (K=74, N=400) ----
                m1 = m1s[k % 2]
                for i in range(2):
                    pp = 16 * (php % 2) + 2 * (k % 8) + i   # pair in pair-tile
                    nc.tensor.matmul(
                        m1[0:80, 512 * i:512 * i + 400],
                        wg2_t[0:74, 80 * pp:80 * pp + 80],
                        ubt2_t[0:74, 400 * pp:400 * pp + 400],
                        start=True, stop=True)
                # ---- tanh(z1/2) over 4 batches ----
                h1_t = h1p.tile([80, 800], BF16, tag="h1")
                nc.scalar.activation(
                    out=h1_t.rearrange("p (u c) -> p u c", u=2),
                    in_=m1.rearrange("p (u c) -> p u c", u=2)[:, :, 0:400],
                    func=AF.Tanh, scale=0.5)
                h1s[k] = h1_t

            if 0 <= kk - 1 < NK:
                q = kk - 1
                # ---- mm2: shared weights, 2 col-tiled instances ----
                h1_t = h1s.pop(q)
                m2 = m2s[q % 2]
                nc.tensor.matmul(
                    m2[0:40, 0:400], w2_t, h1_t[0:80, 0:400],
                    start=True, stop=True)
                nc.tensor.matmul(
                    m2[64:104, 0:400], w2_t, h1_t[0:80, 400:800],
                    start=True, stop=True, tile_position=(0, 64))
                # ---- tanh(z2/2 + b2c) over 4 batches ----
                h2_t = h2p.tile([104, 400], BF16, tag="h2")
                nc.scalar.activation(
                    out=h2_t, in_=m2[0:104, 0:400],
                    func=AF.Tanh, bias=b2_t[0:104, :], scale=0.5)
                h2s[q] = h2_t

            if 0 <= kk - 2 < NK:
                r = kk - 2
                # ---- mm3: [104,1] stationaries; N=400 -> 2 batches ----
                # batch u (in the 8-batch scatter set) = 4*(r%2)+j ->
                # psum (partition slot 32*(u//2), col 200*(u%2))
                h2_t = h2s.pop(r)
                m3 = m3s[(r // 2) % 2]
                for half in range(2):
                    slot = 2 * (r % 2) + half
                    nc.tensor.matmul(
                        m3[32 * slot:32 * slot + 1, 0:400],
                        w3_t[0:104, half:half + 1],
                        h2_t[0:104, 0:400],
                        start=True, stop=True,
                        tile_position=(0, 32 * slot))
                if r % 2 == 1:
                    # ---- scores + b3: PSUM -> SBUF on DVE, scatter ----
                    sc_t = scp.tile([128, 400], F32, tag="sc")
                    nc.vector.tensor_scalar_add(sc_t, m3, b3_t)
                    g, php = r // 32, (r // 8) % 4
                    base = 32 * php + 8 * ((r % 8) // 2)
                    scg_t = scgs[g]
                    nc.sync.dma_start(
                        out=bass.AP(tensor=scg_t.tensor,
                                    offset=scg_t.offset + base * 200,
                                    ap=[[200, 8], [1, 200]]),
                        in_=bass.AP(tensor=sc_t.tensor,
                                    offset=sc_t.offset,
                                    ap=[[32 * 400, 4], [200, 2], [1, 200]]))
                    if r % 32 == 31:
                        softmax_wsum(r // 32)

            if wsum_pending and (kk % 2 == 0 or kk >= NK):
                wsum_pending.pop(0)()

    nc.compile()
    return nc


def host_prep(query_ad, user_behavior, user_behavior_length,
              W1, b1, W2, b2, W3, b3):
    q = np.asarray(query_ad, dtype=np.float32)
    ub = np.asarray(user_behavior, dtype=np.float32)
    lens = np.asarray(user_behavior_length)
    W1 = np.asarray(W1, dtype=np.float32)
    b1 = np.asarray(b1, dtype=np.float32)
    W2 = np.asarray(W2, dtype=np.float32)
    b2 = np.asarray(b2, dtype=np.float32)
    W3 = np.asarray(W3, dtype=np.float32)
    b3 = np.asarray(b3, dtype=np.float32)
    nb = q.shape[0]

    Wa, Wb, Wc, Wd = W1[0:36], W1[36:72], W1[72:108], W1[108:144]
    waug = np.empty((nb, 37, 80), dtype=np.float32)
    waug[:, 0:36, :] = (Wb - Wc)[None, :, :] + q[:, :, None] * Wd[None, :, :]
    waug[:, 36, :] = q @ (Wa + Wc) + b1[None, :]
    waug = waug.astype(BF16NP)

    ubaug = np.empty((nb, 200, 37), dtype=BF16NP)
    ubaug[:, :, 0:36] = ub
    ubaug[:, :, 36] = 1.0

    # sigmoid -> tanh fold: h = 0.5 + 0.5*t with t = tanh(pre/2)
    w2f = (0.5 * W2).astype(BF16NP)                  # [80, 40]
    b2f = 0.5 * (b2 + 0.5 * W2.sum(axis=0))          # [40]
    w3f = 0.5 * W3[:, 0]                             # [40]
    b3f = float(b3[0] + 0.5 * W3.sum())

    # mm3 stationaries select the h2 partition half holding the batch
    w3x = np.zeros((104, 2), dtype=BF16NP)
    w3x[0:40, 0] = w3f
    w3x[64:104, 1] = w3f
    b2c = np.zeros((128, 1), dtype=np.float32)
    b2c[0:40, 0] = b2f
    b2c[64:104, 0] = b2f
    b3c = np.full((128, 1), b3f, dtype=np.float32)

    in_maps = []
    for c in range(N_CORES):
        sl = slice(BC * c, BC * (c + 1))
        # batch-pair K-stacked layouts: rows 0:37 even batch (cols 0:200 of
        # each 400-col pair block), rows 37:74 odd batch (cols 200:400)
        ubaugT = ubaug[sl].transpose(2, 0, 1)        # [37, 512, 200]
        ubt74 = np.zeros((74, BC // 2, 400), dtype=BF16NP)
        ubt74[0:37, :, 0:200] = ubaugT[:, 0::2]
        ubt74[37:74, :, 200:400] = ubaugT[:, 1::2]
        ubt74 = np.ascontiguousarray(ubt74.reshape(74, BC * 200))
        waugT = waug[sl].transpose(1, 0, 2)          # [37, 512, 80]
        wg74 = np.empty((74, BC // 2, 80), dtype=BF16NP)
        wg74[0:37] = waugT[:, 0::2]
        wg74[37:74] = waugT[:, 1::2]
        wg74 = np.ascontiguousarray(wg74.reshape(74, BC * 40))
        # batch-on-partition, e-major copy for the DVE weighted sum
        ubbw = np.ascontiguousarray(
            ub[sl].astype(BF16NP).reshape(N_GRP, 128, 200, 36)
            .transpose(1, 0, 3, 2).reshape(128, N_GRP * 36 * 200))
        lens4 = np.ascontiguousarray(
            lens[sl].astype(np.float32).reshape(N_GRP, 128).T)
        in_maps.append({
            "ubt": ubt74, "wg": wg74,
            "ubbw": ubbw, "lens": lens4,
            "w2": w2f, "w3": w3x, "b2c": b2c, "b3c": b3c,
        })
    return in_maps


_NC_CACHE = {}


def get_module():
    if "nc" not in _NC_CACHE:
        _NC_CACHE["nc"] = build_module()
    return _NC_CACHE["nc"]


def kernel(query_ad, user_behavior, user_behavior_length,
           W1, b1, W2, b2, W3, b3, trace=False):
    nc = get_module()
    in_maps = host_prep(query_ad, user_behavior, user_behavior_length,
                        W1, b1, W2, b2, W3, b3)
    res = run_bass_kernel_spmd(nc, in_maps, core_ids=list(range(N_CORES)),
                               trace=trace)
    outs = [res.results[c]["out"] for c in range(N_CORES)]
    full = np.concatenate(outs, axis=0).reshape(B, 1, 36).astype(np.float32)
    if trace:
        kernel.last_result = res
    return full
